# revision 4
# baseline (speedup 1.0000x reference)
"""Trainium2 Bass kernel for GNN multi-head attention message passing.

Strategy (8 NeuronCores, SPMD, no collectives):
  - Nodes are sharded contiguously: core c owns dst nodes [c*N/8, (c+1)*N/8).
  - Edges are bucketed by destination node-tile (128 dst nodes per tile) on
    the host; each (core, node-tile) group is padded to a uniform number of
    128-edge tiles so the single SPMD program is static.
  - Phase A (per core): compute the full K and V projections locally into an
    interleaved DRAM table KV[node] = [k_row(256) | v_row(256)], plus the
    core-local Q projection.  Linear layers are done as PE matmuls with the
    host supplying transposed activations (lhsT) and W^T (rhs); bias is added
    via a rank-1 matmul with a ones vector.
  - Phase B (per node tile): for each 128-edge tile, one indirect DMA gathers
    the 128 KV rows by src index ([128,1] per-partition index form).  A
    one-hot matrix O[e,n] = (dst_rel[e]==n) built with iota/is_equal drives
    two matmuls: Q-expansion (Q_exp = O^T @ Q_tile, i.e. a PE gather of q[dst])
    and the scatter-add (psum[n,:] += O^T applied to per-edge weighted values).
    Scores use the no-max-subtraction softmax: p = exp(s)/sum(exp(s)) which is
    safe here (|s| <= ~12) and removes the segment-max pass entirely.
    The denominator ride along as a 33rd column per head in the scatter
    matmul.  Finally the output projection runs on-chip via PE transposes.
"""

import math
import sys

sys.path.insert(0, "/opt/trn_rl_repo")

import numpy as np

import concourse.bass as bass
import concourse.tile as tile
from concourse import bacc, mybir
from concourse.bass_utils import run_bass_kernel_spmd

P = 128
D_MODEL = 256
N_HEADS = 8
D_K = 32
N_CORES = 8

F32 = mybir.dt.float32
I32 = mybir.dt.int32


# --------------------------------------------------------------------------
# host-side sharding / index prep
# --------------------------------------------------------------------------

def _prep(inputs, n_nodes, npc):
    """Build per-core input maps. npc = nodes per core."""
    nt = math.ceil(npc / P)            # node tiles per core
    npc_pad = nt * P
    nkv = math.ceil(n_nodes / P) * P   # padded rows of the KV table

    query = np.asarray(inputs["query"], np.float32)
    key = np.asarray(inputs["key"], np.float32)
    value = np.asarray(inputs["value"], np.float32)
    edges = np.asarray(inputs["edges"])
    dst = edges[0].astype(np.int64)
    src = edges[1].astype(np.int64)

    # group edges by (core, node tile)
    core = dst // npc
    dst_l = dst - core * npc
    j = dst_l // P
    group = core * nt + j
    n_groups = N_CORES * nt
    counts = np.bincount(group, minlength=n_groups)
    et = max(1, int(np.ceil(counts.max() / P)))

    order = np.argsort(group, kind="stable")
    g_sorted = group[order]
    starts = np.zeros(n_groups + 1, np.int64)
    np.cumsum(counts, out=starts[1:])
    pos_in_group = np.arange(len(order)) - starts[g_sorted]

    # slot arrays: [n_cores, nt*et, P]
    kv_idx = np.zeros((N_CORES, nt * et, P), np.int32)
    dst_rel = np.full((N_CORES, nt * et, P), -1.0, np.float32)
    e_core = core[order].astype(np.int64)
    e_tile = (j[order] * et + pos_in_group // P).astype(np.int64)
    e_p = (pos_in_group % P).astype(np.int64)
    kv_idx[e_core, e_tile, e_p] = src[order].astype(np.int32)
    dst_rel[e_core, e_tile, e_p] = (dst_l[order] - j[order] * P).astype(np.float32)

    # transposed activations for phase-A lhsT loads
    kp = np.zeros((nkv, D_MODEL), np.float32)
    kp[:n_nodes] = key
    vp = np.zeros((nkv, D_MODEL), np.float32)
    vp[:n_nodes] = value
    keyT = np.ascontiguousarray(kp.T)      # [256, nkv]
    valueT = np.ascontiguousarray(vp.T)

    qp = np.zeros((N_CORES, npc_pad, D_MODEL), np.float32)
    for c in range(N_CORES):
        qp[c, :npc] = query[c * npc:(c + 1) * npc]

    def wt(w):
        t = np.ascontiguousarray(np.asarray(w, np.float32).T)  # [in, out]
        return np.stack([t[:P], t[P:]], 0)                      # [2, 128, 256]

    common = {
        "keyT0": keyT[:P], "keyT1": keyT[P:],
        "valueT0": valueT[:P], "valueT1": valueT[P:],
        "WqT": wt(inputs["Wq"]), "WkT": wt(inputs["Wk"]),
        "WvT": wt(inputs["Wv"]), "WoT": wt(inputs["Wo"]),
        "bq": np.asarray(inputs["bq"], np.float32)[None, :],
        "bk": np.asarray(inputs["bk"], np.float32)[None, :],
        "bv": np.asarray(inputs["bv"], np.float32)[None, :],
        "bo": np.asarray(inputs["bo"], np.float32)[None, :],
        "iota": np.tile(np.arange(P, dtype=np.float32), (P, 1)),
        "ident": np.eye(P, dtype=np.float32),
    }
    in_maps = []
    for c in range(N_CORES):
        qT = np.ascontiguousarray(qp[c].T)  # [256, npc_pad]
        m = dict(common)
        m["qT0"] = qT[:P]
        m["qT1"] = qT[P:]
        m["kv_idx"] = np.ascontiguousarray(kv_idx[c].T)    # [128, nt*et]
        m["dst_rel"] = np.ascontiguousarray(dst_rel[c].T)  # [128, nt*et]
        in_maps.append(m)
    return in_maps, et


# --------------------------------------------------------------------------
# bass program
# --------------------------------------------------------------------------

def _build(n_nodes, npc, et):
    nt = math.ceil(npc / P)
    npc_pad = nt * P
    nkv = math.ceil(n_nodes / P) * P
    nkt = nkv // P
    GA = 8

    nc = bacc.Bacc(None)

    keyT0 = nc.dram_tensor("keyT0", [P, nkv], F32, kind="ExternalInput")
    keyT1 = nc.dram_tensor("keyT1", [P, nkv], F32, kind="ExternalInput")
    valueT0 = nc.dram_tensor("valueT0", [P, nkv], F32, kind="ExternalInput")
    valueT1 = nc.dram_tensor("valueT1", [P, nkv], F32, kind="ExternalInput")
    qT0 = nc.dram_tensor("qT0", [P, npc_pad], F32, kind="ExternalInput")
    qT1 = nc.dram_tensor("qT1", [P, npc_pad], F32, kind="ExternalInput")
    WqT = nc.dram_tensor("WqT", [2, P, D_MODEL], F32, kind="ExternalInput")
    WkT = nc.dram_tensor("WkT", [2, P, D_MODEL], F32, kind="ExternalInput")
    WvT = nc.dram_tensor("WvT", [2, P, D_MODEL], F32, kind="ExternalInput")
    WoT = nc.dram_tensor("WoT", [2, P, D_MODEL], F32, kind="ExternalInput")
    bq = nc.dram_tensor("bq", [1, D_MODEL], F32, kind="ExternalInput")
    bk = nc.dram_tensor("bk", [1, D_MODEL], F32, kind="ExternalInput")
    bv = nc.dram_tensor("bv", [1, D_MODEL], F32, kind="ExternalInput")
    bo = nc.dram_tensor("bo", [1, D_MODEL], F32, kind="ExternalInput")
    iota_d = nc.dram_tensor("iota", [P, P], F32, kind="ExternalInput")
    ident_d = nc.dram_tensor("ident", [P, P], F32, kind="ExternalInput")
    kvidx_d = nc.dram_tensor("kv_idx", [P, nt * et], I32, kind="ExternalInput")
    dstrel_d = nc.dram_tensor("dst_rel", [P, nt * et], F32, kind="ExternalInput")

    out_d = nc.dram_tensor("out", [npc_pad, D_MODEL], F32, kind="ExternalOutput")

    KV = nc.dram_tensor("KV", [nkv, 2 * D_MODEL], F32)
    Q = nc.dram_tensor("Qproj", [npc_pad, D_MODEL], F32)

    mm = nc.tensor.matmul
    ALU = mybir.AluOpType
    AF = mybir.ActivationFunctionType

    with tile.TileContext(nc) as tc:
        with tc.tile_pool(name="res", bufs=1) as res:
            # resident constants
            w_sb = {}
            for name, dram in (("q", WqT), ("k", WkT), ("v", WvT), ("o", WoT)):
                t = res.tile([P, 2, D_MODEL], F32, tag=f"W{name}")
                nc.sync.dma_start(out=t[:, 0, :], in_=dram[0])
                nc.sync.dma_start(out=t[:, 1, :], in_=dram[1])
                w_sb[name] = t
            b_sb = {}
            for name, dram in (("q", bq), ("k", bk), ("v", bv), ("o", bo)):
                t = res.tile([1, D_MODEL], F32, tag=f"b{name}")
                nc.sync.dma_start(out=t[:], in_=dram[:])
                b_sb[name] = t
            iota_sb = res.tile([P, P], F32)
            nc.sync.dma_start(out=iota_sb[:], in_=iota_d[:])
            ident_sb = res.tile([P, P], F32)
            nc.sync.dma_start(out=ident_sb[:], in_=ident_d[:])
            kvidx_sb = res.tile([P, nt * et], I32)
            nc.sync.dma_start(out=kvidx_sb[:], in_=kvidx_d[:])
            dstrel_sb = res.tile([P, nt * et], F32)
            nc.sync.dma_start(out=dstrel_sb[:], in_=dstrel_d[:])
            ones1 = res.tile([1, P], F32)
            nc.vector.memset(ones1[:], 1.0)

            # ---------------- phase A: projections ----------------
            def proj_tiles(x0_d, x1_d, w, b, out_dram, n_tiles, tag):
                with tc.tile_pool(name=f"A{tag}", bufs=2) as ap_, \
                     tc.tile_pool(name=f"Aps{tag}", bufs=2, space="PSUM") as psp, \
                     tc.tile_pool(name=f"Ao{tag}", bufs=3) as op_:
                    for g in range(0, n_tiles, GA):
                        ntl = min(GA, n_tiles - g)
                        x0 = ap_.tile([P, GA * P], F32, tag="x0")
                        x1 = ap_.tile([P, GA * P], F32, tag="x1")
                        nc.sync.dma_start(out=x0[:, :ntl * P],
                                          in_=x0_d[:, g * P:(g + ntl) * P])
                        nc.sync.dma_start(out=x1[:, :ntl * P],
                                          in_=x1_d[:, g * P:(g + ntl) * P])
                        for t in range(ntl):
                            ps = psp.tile([P, D_MODEL], F32, tag="ps")
                            mm(ps[:], lhsT=x0[:, t * P:(t + 1) * P], rhs=w[:, 0, :],
                               start=True, stop=False)
                            mm(ps[:], lhsT=x1[:, t * P:(t + 1) * P], rhs=w[:, 1, :],
                               start=False, stop=False)
                            mm(ps[:], lhsT=ones1[:], rhs=b[:], start=False, stop=True)
                            o = op_.tile([P, D_MODEL], F32, tag="o")
                            if t % 2 == 0:
                                nc.scalar.copy(o[:], ps[:])
                            else:
                                nc.vector.tensor_copy(o[:], ps[:])
                            r0 = (g + t) * P
                            nc.sync.dma_start(out=out_dram[r0:r0 + P, :], in_=o[:])

            # K and V interleave into the same KV table columns
            def proj_kv():
                with tc.tile_pool(name="Akv", bufs=2) as ap_, \
                     tc.tile_pool(name="Akvps", bufs=2, space="PSUM") as psp, \
                     tc.tile_pool(name="Akvo", bufs=3) as op_:
                    for g in range(0, nkt, GA):
                        ntl = min(GA, nkt - g)
                        k0 = ap_.tile([P, GA * P], F32, tag="k0")
                        k1 = ap_.tile([P, GA * P], F32, tag="k1")
                        v0 = ap_.tile([P, GA * P], F32, tag="v0")
                        v1 = ap_.tile([P, GA * P], F32, tag="v1")
                        sl = slice(g * P, (g + ntl) * P)
                        nc.sync.dma_start(out=k0[:, :ntl * P], in_=keyT0[:, sl])
                        nc.sync.dma_start(out=k1[:, :ntl * P], in_=keyT1[:, sl])
                        nc.sync.dma_start(out=v0[:, :ntl * P], in_=valueT0[:, sl])
                        nc.sync.dma_start(out=v1[:, :ntl * P], in_=valueT1[:, sl])
                        for t in range(ntl):
                            tsl = slice(t * P, (t + 1) * P)
                            psk = psp.tile([P, D_MODEL], F32, tag="psk")
                            mm(psk[:], lhsT=k0[:, tsl], rhs=w_sb["k"][:, 0, :],
                               start=True, stop=False)
                            mm(psk[:], lhsT=k1[:, tsl], rhs=w_sb["k"][:, 1, :],
                               start=False, stop=False)
                            mm(psk[:], lhsT=ones1[:], rhs=b_sb["k"][:],
                               start=False, stop=True)
                            psv = psp.tile([P, D_MODEL], F32, tag="psv")
                            mm(psv[:], lhsT=v0[:, tsl], rhs=w_sb["v"][:, 0, :],
                               start=True, stop=False)
                            mm(psv[:], lhsT=v1[:, tsl], rhs=w_sb["v"][:, 1, :],
                               start=False, stop=False)
                            mm(psv[:], lhsT=ones1[:], rhs=b_sb["v"][:],
                               start=False, stop=True)
                            o = op_.tile([P, 2 * D_MODEL], F32, tag="kv")
                            nc.scalar.copy(o[:, :D_MODEL], psk[:])
                            nc.vector.tensor_copy(o[:, D_MODEL:], psv[:])
                            r0 = (g + t) * P
                            nc.sync.dma_start(out=KV[r0:r0 + P, :], in_=o[:])

            proj_kv()
            proj_tiles(qT0, qT1, w_sb["q"], b_sb["q"], Q, nt, "q")

            # ---------------- phase B: edge processing ----------------
            inv_sqrt_dk = 1.0 / math.sqrt(D_K)
            with tc.tile_pool(name="Bq", bufs=2) as qp_, \
                 tc.tile_pool(name="Bkv", bufs=2) as kvp, \
                 tc.tile_pool(name="Boh", bufs=3) as ohp, \
                 tc.tile_pool(name="Bsm", bufs=3) as smp, \
                 tc.tile_pool(name="Bfin", bufs=2) as fin, \
                 tc.tile_pool(name="BpsQ", bufs=2, space="PSUM") as psq_p, \
                 tc.tile_pool(name="BpsNT", bufs=2, space="PSUM") as psnt_p, \
                 tc.tile_pool(name="BpsOT", bufs=2, space="PSUM") as psot_p, \
                 tc.tile_pool(name="BpsFin", bufs=1, space="PSUM") as psfin_p:
                for j in range(nt):
                    qtile = qp_.tile([P, D_MODEL], F32, tag="qtile")
                    nc.sync.dma_start(out=qtile[:], in_=Q[j * P:(j + 1) * P, :])
                    kvg = kvp.tile([P, et, 2 * D_MODEL], F32, tag="kvg")
                    for e in range(et):
                        nc.gpsimd.indirect_dma_start(
                            out=kvg[:, e, :], out_offset=None, in_=KV[:],
                            in_offset=bass.IndirectOffsetOnAxis(
                                ap=kvidx_sb[:, j * et + e:j * et + e + 1], axis=0),
                        )
                    psnt = psnt_p.tile([P, N_HEADS * (D_K + 1)], F32, tag="psnt")
                    for e in range(et):
                        col = j * et + e
                        O = ohp.tile([P, P], F32, tag="O")
                        nc.vector.tensor_scalar(
                            out=O[:], in0=iota_sb[:],
                            scalar1=dstrel_sb[:, col:col + 1], scalar2=None,
                            op0=ALU.is_equal)
                        psot = psot_p.tile([P, P], F32, tag="psot")
                        nc.tensor.transpose(out=psot[:], in_=O[:],
                                            identity=ident_sb[:])
                        OT = ohp.tile([P, P], F32, tag="OT")
                        nc.scalar.copy(OT[:], psot[:])
                        psq = psq_p.tile([P, D_MODEL], F32, tag="psq")
                        mm(psq[:], lhsT=OT[:], rhs=qtile[:], start=True, stop=True)
                        prod = smp.tile([P, D_MODEL], F32, tag="prod")
                        nc.vector.tensor_tensor(
                            out=prod[:], in0=psq[:], in1=kvg[:, e, :D_MODEL],
                            op=ALU.mult)
                        s_et = smp.tile([P, N_HEADS], F32, tag="s")
                        nc.vector.tensor_reduce(
                            out=s_et[:],
                            in_=prod[:].rearrange("p (h k) -> p h k", h=N_HEADS),
                            axis=mybir.AxisListType.X, op=ALU.add)
                        ex = smp.tile([P, N_HEADS], F32, tag="ex")
                        nc.scalar.activation(ex[:], s_et[:], AF.Exp,
                                             scale=inv_sqrt_dk)
                        rhs_t = smp.tile([P, N_HEADS, D_K + 1], F32, tag="rhs")
                        nc.vector.tensor_tensor(
                            out=rhs_t[:, :, :D_K],
                            in0=kvg[:, e, D_MODEL:].rearrange(
                                "p (h k) -> p h k", h=N_HEADS),
                            in1=ex[:].to_broadcast([P, N_HEADS, D_K]),
                            op=ALU.mult)
                        nc.vector.tensor_copy(rhs_t[:, :, D_K], ex[:])
                        mm(psnt[:], lhsT=O[:],
                           rhs=rhs_t[:].rearrange("p h k -> p (h k)"),
                           start=(e == 0), stop=(e == et - 1))
                    # normalize + output projection
                    psnt3 = psnt[:].rearrange("p (h k) -> p h k", k=D_K + 1)
                    den = fin.tile([P, N_HEADS], F32, tag="den")
                    nc.vector.tensor_scalar(
                        out=den[:], in0=psnt3[:, :, D_K], scalar1=1e-30,
                        scalar2=None, op0=ALU.add)
                    rcp = fin.tile([P, N_HEADS], F32, tag="rcp")
                    nc.vector.reciprocal(out=rcp[:], in_=den[:])
                    nor = fin.tile([P, D_MODEL], F32, tag="nor")
                    nc.vector.tensor_tensor(
                        out=nor[:].rearrange("p (h k) -> p h k", h=N_HEADS),
                        in0=psnt3[:, :, :D_K],
                        in1=rcp[:].to_broadcast([P, N_HEADS, D_K]),
                        op=ALU.mult)
                    pst = psfin_p.tile([P, 2, P], F32, tag="pst")
                    nc.tensor.transpose(out=pst[:, 0, :], in_=nor[:, :P],
                                        identity=ident_sb[:])
                    nc.tensor.transpose(out=pst[:, 1, :], in_=nor[:, P:],
                                        identity=ident_sb[:])
                    t01 = fin.tile([P, 2, P], F32, tag="t01")
                    nc.scalar.copy(t01[:], pst[:])
                    pso = psfin_p.tile([P, D_MODEL], F32, tag="pso")
                    mm(pso[:], lhsT=t01[:, 0, :], rhs=w_sb["o"][:, 0, :],
                       start=True, stop=False)
                    mm(pso[:], lhsT=t01[:, 1, :], rhs=w_sb["o"][:, 1, :],
                       start=False, stop=False)
                    mm(pso[:], lhsT=ones1[:], rhs=b_sb["o"][:],
                       start=False, stop=True)
                    ob = fin.tile([P, D_MODEL], F32, tag="ob")
                    nc.scalar.copy(ob[:], pso[:])
                    nc.sync.dma_start(out=out_d[j * P:(j + 1) * P, :], in_=ob[:])

    nc.finalize()
    return nc


_CACHE = {}


def _get_program(n_nodes, npc, et):
    key = (n_nodes, npc, et)
    if key not in _CACHE:
        _CACHE[key] = _build(n_nodes, npc, et)
    return _CACHE[key]


def kernel(**inputs):
    n_nodes = inputs["query"].shape[0]
    npc = n_nodes // N_CORES
    in_maps, et = _prep(inputs, n_nodes, npc)
    nc = _get_program(n_nodes, npc, et)
    res = run_bass_kernel_spmd(nc, in_maps, list(range(N_CORES)))
    out = np.concatenate(
        [res.results[c]["out"][:npc] for c in range(N_CORES)], axis=0)
    return out.astype(np.float32)
